# revision 16
# baseline (speedup 1.0000x reference)
"""GAT (nn_GAT_29523605193094) Trainium2 kernel.

The reference keeps the source bug ``src, dst = edges[0], edges[0]``, so the
adjacency matrix is purely diagonal: adj[i, i] = (i appears in edges[0]).
After the -inf masking, row i of the [N, N, H] score tensor has exactly one
finite entry (j = i) when node i is covered, so softmax over axis=1 yields
exactly 1.0 at (i, i) and 0.0 elsewhere, and the output row is exactly
h[i] = (X @ W)[i].  Rows for uncovered nodes are all -inf -> softmax is NaN
-> the output row is NaN.  Both cases are reproduced here:

    out = X @ W            (on 8 NeuronCores, row-sharded)
    out[~covered] = NaN    (host-side mask from edges[0])

The device work is a row-sharded [4096, 512] @ [512, 256] matmul.  Each core
gets 512 rows of X.  Inputs are marshalled to bf16 on the host (tolerance is
2e-2; bf16 with fp32 PSUM accumulation lands ~2e-3), which both halves the
HBM traffic and lets the PE run single-pass (fp32 needs the 2x LOW_HIGH
replay).  Layouts are packed so every DMA descriptor covers a full 2-4 KiB
per-partition row, and each transfer is split across both HWDGE queues
(sync + scalar) by partition halves so all 16 DMA engines stream at once.
A short burst of dummy matmuls warms the PE clock out of its low p-state
while the input DMAs are in flight.
"""

import numpy as np

N = 4096
IN = 512
OUT = 256
NCORES = 8
RB = N // NCORES  # 512 rows per core
P = 128
KT = IN // P      # 4 contraction chunks
MT = RB // P      # 4 output row blocks per core
HP = P // 2       # partition half for dual-queue DMA splits
KW = KT * RB      # xb free width (bf16 cols)
WW = KT * OUT     # wb free width
OW = MT * OUT     # outb free width
N_WARM = 12       # PE p-state warmup matmuls

_state = {}

# test.py reads this after a traced call for the HW exec time.
LAST_RESULTS = None


def _build():
    import concourse.mybir as mybir
    import concourse.tile as tile
    from concourse import bacc
    from concourse.bass import ts

    nc = bacc.Bacc(
        "TRN2",
        target_bir_lowering=False,
        debug=False,
        num_devices=NCORES,
    )
    f32 = mybir.dt.float32
    bf16 = mybir.dt.bfloat16
    # xb[p, k*RB + c] = X[core*RB + c, k*128 + p]   (bf16, 4 KiB rows)
    # wb[p, k*OUT + f] = W[k*128 + p, f]            (bf16, 2 KiB rows)
    # outb[p, m*OUT + f] = (X @ W)[core*RB + m*128 + p, f]  (bf16, 2 KiB rows)
    xb = nc.dram_tensor("xb", [P, KW], bf16, kind="ExternalInput")
    wb = nc.dram_tensor("wb", [P, WW], bf16, kind="ExternalInput")
    outb = nc.dram_tensor("outb", [P, OW], bf16, kind="ExternalOutput")

    with tile.TileContext(nc) as tc:
        with (
            tc.tile_pool(name="ins", bufs=1) as in_pool,
            tc.tile_pool(name="warm", bufs=1) as warm_pool,
            tc.tile_pool(name="outs", bufs=4) as out_pool,
            tc.tile_pool(name="ps", bufs=4, space="PSUM") as psum_pool,
            tc.tile_pool(name="psw", bufs=1, space="PSUM") as psw_pool,
        ):
            xb_t = in_pool.tile([P, KW], bf16)
            wb_t = in_pool.tile([P, WW], bf16)
            # Three parallel queues: wb rides the gpsimd SWDGE queue while
            # the two HWDGE queues carry xb split by partition halves.  The
            # DMA engines service each queue's transfers in issue order, so
            # phase A's k0/k1 half of xb goes first and the k2/k3 half
            # streams while phase A computes.
            HK = KW // 2
            HW = WW // 2
            nc.gpsimd.dma_start(wb_t[:, 0:HW], wb[:, 0:HW])
            nc.gpsimd.dma_start(wb_t[:, HW:WW], wb[:, HW:WW])
            nc.sync.dma_start(xb_t[0:HP, 0:HK], xb[0:HP, 0:HK])
            nc.scalar.dma_start(xb_t[HP:P, 0:HK], xb[HP:P, 0:HK])
            nc.sync.dma_start(xb_t[0:HP, HK:KW], xb[0:HP, HK:KW])
            nc.scalar.dma_start(xb_t[HP:P, HK:KW], xb[HP:P, HK:KW])

            # Warm the PE out of its low p-state while the inputs stream in:
            # dummy matmuls on a zeroed tile into a scratch PSUM bank.  The
            # clock ramps with continuous busy time, so the real matmuls
            # below start at speed instead of at 0.65 GHz.
            warm_t = warm_pool.tile([P, P + OUT], bf16)
            nc.vector.memset(warm_t[:], 0.0)
            ps_w = psw_pool.tile([P, OUT], f32, name="psw", tag="psw")
            for _ in range(N_WARM):
                nc.tensor.matmul(
                    ps_w[:], warm_t[:, 0:P], warm_t[:, P : P + OUT],
                    start=True, stop=True,
                )

            # Two k-phases so compute starts as soon as the k0/k1 half of xb
            # lands; each m-block's PSUM completes at its phase-B stop, so
            # its copy + output DMA drain while later blocks compute.  bf16
            # operands stream 1 column/cycle.
            obs = [
                out_pool.tile([P, OUT], bf16, name=f"ob{i}") for i in range(MT)
            ]
            pss = [
                psum_pool.tile([P, OUT], f32, name=f"ps{m}", tag="ps")
                for m in range(MT)
            ]
            for m in range(MT):
                for k in (0, 1):
                    nc.tensor.matmul(
                        pss[m][:],
                        xb_t[:, k * RB + m * P : k * RB + (m + 1) * P],
                        wb_t[:, ts(k, OUT)],
                        start=(k == 0),
                        stop=False,
                    )
            for m in range(MT):
                for k in (2, 3):
                    nc.tensor.matmul(
                        pss[m][:],
                        xb_t[:, k * RB + m * P : k * RB + (m + 1) * P],
                        wb_t[:, ts(k, OUT)],
                        start=False,
                        stop=(k == KT - 1),
                    )
                # Drain each m-block the moment it completes, split across
                # both queues by partition half, so only the last 64 KiB
                # block's transfer is exposed after the final matmul.
                nc.vector.tensor_copy(obs[m][:], pss[m][:])
                nc.sync.dma_start(outb[0:HP, ts(m, OUT)], obs[m][0:HP, :])
                nc.scalar.dma_start(outb[HP:P, ts(m, OUT)], obs[m][HP:P, :])

    nc.compile()
    return nc


def kernel(X, edges, W, A):
    global LAST_RESULTS
    import ml_dtypes
    from concourse.bass_utils import run_bass_kernel_spmd

    X = np.asarray(X, dtype=np.float32)
    W = np.asarray(W, dtype=np.float32)
    edges = np.asarray(edges)

    if "nc" not in _state:
        _state["nc"] = _build()
    nc = _state["nc"]

    bf16 = ml_dtypes.bfloat16
    XT = X.T  # [IN, N]
    wb_np = np.ascontiguousarray(
        W.reshape(KT, P, OUT).transpose(1, 0, 2).reshape(P, WW)
    ).astype(bf16)
    in_maps = []
    for c in range(NCORES):
        shard = XT[:, c * RB : (c + 1) * RB]  # [IN, RB]
        xb_np = np.ascontiguousarray(
            shard.reshape(KT, P, RB).transpose(1, 0, 2).reshape(P, KW)
        ).astype(bf16)
        in_maps.append({"xb": xb_np, "wb": wb_np})

    # The device occasionally reports a transient NRT_EXEC_UNIT_UNRECOVERABLE
    # on an otherwise-good kernel; retry before giving up.
    last_exc = None
    for _attempt in range(3):
        try:
            res = run_bass_kernel_spmd(nc, in_maps, core_ids=list(range(NCORES)))
            break
        except Exception as exc:  # noqa: BLE001
            last_exc = exc
            import time

            time.sleep(2.0)
    else:
        raise last_exc
    LAST_RESULTS = res
    out = np.concatenate(
        [
            np.asarray(res.results[c]["outb"])
            .astype(np.float32)
            .reshape(P, MT, OUT)
            .transpose(1, 0, 2)
            .reshape(RB, OUT)
            for c in range(NCORES)
        ],
        axis=0,
    )

    # Reference semantics: nodes absent from edges[0] have an all -inf score
    # row; softmax of that is NaN, which propagates to the output row.
    covered = np.zeros(N, dtype=bool)
    covered[edges[0]] = True
    if not covered.all():
        out[~covered] = np.nan
    return out
